# revision 1
# baseline (speedup 1.0000x reference)
"""JSD loss kernel for Trainium2 (8 NeuronCores, row-sharded SPMD).

loss[r] = beta*KL(P||M) + (1-beta)*KL(Q||M), beta=0.5, M=(P+Q)/2
        = sum_v [ p*lp + q*lq - s*log(m) ] * 0.5      (s = p+q, m = s/2)

Per-element plan (per core: 256 rows x 32000 vocab):
  ACT:  p' = Exp(lp - ln2) = p/2,  q' = Exp(lq - ln2) = q/2   (f32)
  PE :  m  = p' + q'  via two identity matmuls accumulating in PSUM
  ACT:  logm = Ln(m)                                          (f32)
  DVE:  TTR-AB: accum = sum( [lp|lq] * [p'|q'] )  = 0.5*(A+B)
        TTR-C : accum = sum( m * logm )           = 0.5*C
        chunk loss = AB_acc - C_acc   (exactly the 0.5-scaled JSD sum)
Chunked partial sums (per 2048-col chunk) keep the big-sum cancellation
error at the ~1e-5 level.
"""

import sys
from contextlib import ExitStack

import numpy as np

sys.path.insert(0, "/opt/trn_rl_repo")

N, V = 2048, 32000
NCORES = 8
R = N // NCORES  # rows per core = 256
P = 128  # partitions
NBLK = R // P  # row blocks per core = 2
CHUNKS = [2048] * 15 + [1280]  # 15*2048 + 1280 = 32000, PSUM-bank aligned
LN2 = 0.6931471805599453

_CACHE = {}


def _build_program():
    import concourse.bacc as bacc
    import concourse.tile as tile
    from concourse import mybir

    nc = bacc.Bacc(
        "TRN2",
        target_bir_lowering=False,
        debug=False,
        enable_asserts=False,
        num_devices=1,
    )
    lp_d = nc.dram_tensor("log_p", [R, V], mybir.dt.float32, kind="ExternalInput")
    lq_d = nc.dram_tensor("log_q", [R, V], mybir.dt.float32, kind="ExternalInput")
    id_d = nc.dram_tensor("ident", [P, P], mybir.dt.float32, kind="ExternalInput")
    out_d = nc.dram_tensor("loss", [R, 1], mybir.dt.float32, kind="ExternalOutput")

    lp = lp_d.ap()
    lq = lq_d.ap()
    out = out_d.ap()

    fp32 = mybir.dt.float32
    bf16 = mybir.dt.bfloat16
    Exp = mybir.ActivationFunctionType.Exp
    Ln = mybir.ActivationFunctionType.Ln
    mult = mybir.AluOpType.mult
    add = mybir.AluOpType.add

    with tile.TileContext(nc) as tc, ExitStack() as ctx:
        const = ctx.enter_context(tc.tile_pool(name="const", bufs=1))
        loads = ctx.enter_context(tc.tile_pool(name="loads", bufs=4))
        acts = ctx.enter_context(tc.tile_pool(name="acts", bufs=4))
        logms = ctx.enter_context(tc.tile_pool(name="logms", bufs=2))
        scr = ctx.enter_context(tc.tile_pool(name="scr", bufs=2))
        accs = ctx.enter_context(tc.tile_pool(name="accs", bufs=8))
        parts = ctx.enter_context(tc.tile_pool(name="parts", bufs=2))
        outs = ctx.enter_context(tc.tile_pool(name="outs", bufs=2))
        psum = ctx.enter_context(tc.tile_pool(name="psum", bufs=2, space="PSUM"))

        ident_sb = const.tile([P, P], fp32)
        nc.sync.dma_start(out=ident_sb[:], in_=id_d.ap())
        neg_ln2 = const.tile([P, 1], fp32)
        nc.vector.memset(neg_ln2[:], -LN2)

        nch = len(CHUNKS)
        for b in range(NBLK):
            r0 = b * P
            ab_parts = parts.tile([P, nch], fp32, tag="abp")
            c_parts = parts.tile([P, nch], fp32, tag="cp")
            for i, C in enumerate(CHUNKS):
                c0 = sum(CHUNKS[:i])
                lplq = loads.tile([P, 2 * 2048], fp32, tag="lplq")
                pq = acts.tile([P, 2 * 2048], fp32, tag="pq")
                nc.sync.dma_start(
                    out=lplq[:, 0:C], in_=lp[r0 : r0 + P, c0 : c0 + C]
                )
                nc.sync.dma_start(
                    out=lplq[:, C : 2 * C], in_=lq[r0 : r0 + P, c0 : c0 + C]
                )
                # p' = exp(lp - ln2) = p/2 ; q' = q/2
                nc.scalar.activation(
                    out=pq[:, 0:C], in_=lplq[:, 0:C], func=Exp, bias=neg_ln2[:]
                )
                nc.scalar.activation(
                    out=pq[:, C : 2 * C], in_=lplq[:, C : 2 * C], func=Exp, bias=neg_ln2[:]
                )
                # m = p' + q'  (PE identity matmuls accumulate into PSUM)
                m_ps = psum.tile([P, 2048], fp32, tag="m")
                for j0 in range(0, C, 512):
                    w = min(512, C - j0)
                    nc.tensor.matmul(
                        out=m_ps[:, j0 : j0 + w],
                        lhsT=ident_sb[:],
                        rhs=pq[:, j0 : j0 + w],
                        start=True,
                        stop=False,
                    )
                    nc.tensor.matmul(
                        out=m_ps[:, j0 : j0 + w],
                        lhsT=ident_sb[:],
                        rhs=pq[:, C + j0 : C + j0 + w],
                        start=False,
                        stop=True,
                    )
                logm = logms.tile([P, 2048], fp32, tag="logm")
                nc.scalar.activation(out=logm[:, 0:C], in_=m_ps[:, 0:C], func=Ln)

                junk = scr.tile([P, 2 * 2048], fp32, tag="junk")
                # AB: sum over both halves of lplq*pq = 0.5*(A+B)
                nc.vector.scalar_tensor_tensor(
                    out=junk[:, 0 : 2 * C],
                    in0=lplq[:, 0 : 2 * C],
                    scalar=1.0,
                    in1=pq[:, 0 : 2 * C],
                    op0=mult,
                    op1=mult,
                    accum_out=ab_parts[:, i : i + 1],
                )
                # C: sum m*logm = 0.5*C
                nc.vector.scalar_tensor_tensor(
                    out=junk[:, 0:C],
                    in0=logm[:, 0:C],
                    scalar=1.0,
                    in1=m_ps[:, 0:C],
                    op0=mult,
                    op1=mult,
                    accum_out=c_parts[:, i : i + 1],
                )
            d_parts = parts.tile([P, nch], fp32, tag="dp")
            nc.vector.tensor_sub(d_parts[:], ab_parts[:], c_parts[:])
            loss_b = outs.tile([P, 1], fp32)
            nc.vector.reduce_sum(
                out=loss_b[:], in_=d_parts[:], axis=mybir.AxisListType.X
            )
            nc.sync.dma_start(out=out[r0 : r0 + P, :], in_=loss_b[:])

    nc.compile()
    return nc


def _get_program():
    if "nc" not in _CACHE:
        _CACHE["nc"] = _build_program()
    return _CACHE["nc"]


def kernel(log_q: np.ndarray, log_p: np.ndarray, _trace: bool = False):
    from concourse.bass_utils import run_bass_kernel_spmd

    log_q = np.ascontiguousarray(np.asarray(log_q, dtype=np.float32))
    log_p = np.ascontiguousarray(np.asarray(log_p, dtype=np.float32))
    assert log_q.shape == (N, V) and log_p.shape == (N, V)

    nc = _get_program()
    ident = np.eye(P, dtype=np.float32)
    in_maps = []
    for c in range(NCORES):
        sl = slice(c * R, (c + 1) * R)
        in_maps.append(
            {"log_p": log_p[sl], "log_q": log_q[sl], "ident": ident}
        )
    res = run_bass_kernel_spmd(
        nc, in_maps, core_ids=list(range(NCORES)), trace=_trace
    )
    _CACHE["last_results"] = res
    outs = [res.results[c]["loss"].reshape(R) for c in range(NCORES)]
    return np.concatenate(outs, axis=0).astype(np.float32)

